# revision 4
# baseline (speedup 1.0000x reference)
"""Trainium2 Bass kernel for nn_BaseAggregator_31439160607279.

Math (reference):
  af (a,c,f,t), imf (v,c,h,w), split c into k=2 heads of 256 ch.
  sims[a,v,k,hw,t] = sum_c af*imf ; + cls[a,v,k] ; relu ; max over hw ;
  masked mean over t (mask m[a,t] in {0,1}, den = f*sum_t m) ; sum over k.

Strategy:
  - Shard the audio dim a=32 across 8 cores (4 audios/core); images replicated.
  - Pack (a_local, t) pairs into the matmul M dim, keeping only mask-active t
    (m=0 columns contribute nothing to the masked sum).
  - Big matmuls in float32r (fp32 @ ~FP22, full PE rate at N>=256):
      lhsT = af[a,k] channel-chunk (K=128, M=128 packed rows)
      rhs  = [imf[v0,k] | imf[v1,k]] (K=128, N=392), accumulate 2 chunks.
  - relu(max_hw(x)+cls) == max_hw(relu(x+cls)): reduce_max on raw PSUM,
    then add per-(row,a)-broadcast cls (one-hot matmul) and relu.
  - Masked t-sum via matmul with block-diagonal mask columns (K=packed rows,
    M=4 audios), accumulated over M-tiles; divide by den, sum heads.
"""

import math
from contextlib import ExitStack

import numpy as np

import concourse.bacc as bacc
import concourse.mybir as mybir
import concourse.tile as tile
from concourse.bass_utils import run_bass_kernel_spmd

# Problem dims (hardcoded per spec)
A, V, C, F, T, H, W = 32, 32, 512, 1, 200, 14, 14
K = 2                    # heads
NCH = C // K             # 256 channels per head
KC = 2                   # channel chunks per head
KP = NCH // KC           # 128 = contraction per matmul
HW = H * W               # 196
NCORES = 8
AL = A // NCORES         # 4 local audios per core
NVP = V // 2             # 16 image pairs
NPAIR = 2 * HW           # 392 = matmul free dim

GATHER = True            # pack only mask-active (a, t) rows
DMA_G = 4                # image pairs per DMA chunk

TRACE = False            # set True (e.g. from test.py) to profile
LAST_RESULTS = None      # BassKernelResults of the last run

_kernel_cache = {}

f32 = mybir.dt.float32
f32r = mybir.dt.float32r
X = mybir.AxisListType.X


def _build(MT: int):
    """Build + compile the per-core Bass program for MT packed-row tiles."""
    nc = bacc.Bacc("TRN2", target_bir_lowering=False, debug=False)

    afp_d = nc.dram_tensor("afp", (K, KC, KP, MT * 128), f32r, kind="ExternalInput")
    imf_d = nc.dram_tensor("imf", (K, KC, KP, V * HW), f32r, kind="ExternalInput")
    acls_d = nc.dram_tensor("acls", (K, KC, KP, AL), f32, kind="ExternalInput")
    icls_d = nc.dram_tensor("icls", (K, KC, KP, V), f32, kind="ExternalInput")
    onehot_d = nc.dram_tensor("onehot", (MT, AL, 128), f32, kind="ExternalInput")
    maskc_d = nc.dram_tensor("maskc", (MT, 128, AL), f32, kind="ExternalInput")
    maskf_d = nc.dram_tensor("maskf", (AL, T), f32, kind="ExternalInput")
    outk_d = nc.dram_tensor("outk", (AL, K * V), f32, kind="ExternalOutput")
    outsum_d = nc.dram_tensor("outsum", (AL, V), f32, kind="ExternalOutput")

    with tile.TileContext(nc) as tc, ExitStack() as ctx:
        cst = ctx.enter_context(tc.tile_pool(name="cst", bufs=1))
        ps_big = ctx.enter_context(tc.tile_pool(name="ps_big", bufs=5, space="PSUM"))
        ps_sm = ctx.enter_context(tc.tile_pool(name="ps_sm", bufs=2, space="PSUM"))

        # --- persistent SBUF tiles + input DMAs ---
        afp_sb, imf_sb, acls_sb, icls_sb = {}, {}, {}, {}
        for k in range(K):
            for kc in range(KC):
                ta = cst.tile([KP, MT * 128], f32r, tag=f"afp{k}{kc}", name=f"afp{k}{kc}")
                nc.sync.dma_start(out=ta[:], in_=afp_d.ap()[k, kc])
                afp_sb[k, kc] = ta
                tac = cst.tile([KP, AL], f32, tag=f"acls{k}{kc}", name=f"acls{k}{kc}")
                nc.sync.dma_start(out=tac[:], in_=acls_d.ap()[k, kc])
                acls_sb[k, kc] = tac
                tic = cst.tile([KP, V], f32, tag=f"icls{k}{kc}", name=f"icls{k}{kc}")
                nc.sync.dma_start(out=tic[:], in_=icls_d.ap()[k, kc])
                icls_sb[k, kc] = tic

        onehot_sb, maskc_sb = [], []
        for mt in range(MT):
            toh = cst.tile([AL, 128], f32, tag=f"onehot{mt}", name=f"onehot{mt}")
            nc.sync.dma_start(out=toh[:], in_=onehot_d.ap()[mt])
            onehot_sb.append(toh)
            tmc = cst.tile([128, AL], f32, tag=f"maskc{mt}", name=f"maskc{mt}")
            nc.sync.dma_start(out=tmc[:], in_=maskc_d.ap()[mt])
            maskc_sb.append(tmc)
        maskf_sb = cst.tile([AL, T], f32, tag="maskf", name="maskf_sb")
        nc.sync.dma_start(out=maskf_sb[:], in_=maskf_d.ap())

        # image features: chunked DMAs so compute can start early
        for k in range(K):
            for kc in range(KC):
                imf_sb[k, kc] = cst.tile([KP, V * HW], f32r, tag=f"imf{k}{kc}", name=f"imf{k}{kc}")
        for k in range(K):
            for g in range(NVP // DMA_G):
                sl = slice(g * DMA_G * NPAIR, (g + 1) * DMA_G * NPAIR)
                for kc in range(KC):
                    nc.sync.dma_start(
                        out=imf_sb[k, kc][:, sl], in_=imf_d.ap()[k, kc][:, sl]
                    )

        # --- cls_sims: cls[a, k*V+v] = sum_c acls*icls ---
        cls_sb = cst.tile([AL, K * V], f32, tag="cls", name="cls_sb")
        for k in range(K):
            pc = ps_sm.tile([AL, V], f32, tag="ps_sm", name="ps_cls")
            for kc in range(KC):
                nc.tensor.matmul(
                    pc[:], lhsT=acls_sb[k, kc][:], rhs=icls_sb[k, kc][:],
                    start=(kc == 0), stop=(kc == 1),
                )
            nc.vector.tensor_copy(cls_sb[:, k * V:(k + 1) * V], pc[:])

        # --- main loop: sims matmuls + max over hw ---
        smraw = [cst.tile([128, K * V], f32, tag=f"smraw{mt}", name=f"smraw{mt}") for mt in range(MT)]
        for k in range(K):
            for vp in range(NVP):
                rsl = slice(vp * NPAIR, (vp + 1) * NPAIR)
                col = k * V + vp * 2
                for mt in range(MT):
                    ps = ps_big.tile([128, NPAIR], f32, tag="ps_big", name="ps_sims")
                    for kc in range(KC):
                        nc.tensor.matmul(
                            ps[:],
                            lhsT=afp_sb[k, kc][:, mt * 128:(mt + 1) * 128],
                            rhs=imf_sb[k, kc][:, rsl],
                            start=(kc == 0), stop=(kc == 1),
                        )
                    nc.vector.reduce_max(
                        smraw[mt][:, col:col + 2],
                        ps[:].rearrange("p (i x) -> p i x", i=2),
                        axis=X,
                    )

        # --- per M-tile: +cls, relu, masked row-sum ---
        num_sb = cst.tile([AL, K * V], f32, tag="num", name="num_sb")
        nc.vector.memset(num_sb[:], 0.0)
        sm_pool = ctx.enter_context(tc.tile_pool(name="sm", bufs=2))
        for mt in range(MT):
            pb = ps_sm.tile([128, K * V], f32, tag="ps_sm", name="ps_bcast")
            nc.tensor.matmul(pb[:], lhsT=onehot_sb[mt][:], rhs=cls_sb[:],
                             start=True, stop=True)
            sm2 = sm_pool.tile([128, K * V], f32, tag="sm2", name="sm2")
            nc.vector.tensor_add(sm2[:], smraw[mt][:], pb[:])
            nc.vector.tensor_scalar_max(sm2[:], sm2[:], 0.0)
            pn = ps_sm.tile([AL, K * V], f32, tag="ps_sm", name="ps_num")
            nc.tensor.matmul(pn[:], lhsT=maskc_sb[mt][:], rhs=sm2[:],
                             start=True, stop=True)
            nc.vector.tensor_add(num_sb[:], num_sb[:], pn[:])

        # --- den, divide, head-sum, out ---
        den = cst.tile([AL, 1], f32, tag="den", name="den")
        nc.vector.reduce_sum(den[:], maskf_sb[:], axis=X)
        rden = cst.tile([AL, 1], f32, tag="rden", name="rden")
        nc.vector.reciprocal(rden[:], den[:])
        outk_sb = cst.tile([AL, K * V], f32, tag="outk", name="outk_sb")
        nc.vector.tensor_scalar_mul(outk_sb[:], num_sb[:], rden[:])
        outsum_sb = cst.tile([AL, V], f32, tag="outsum", name="outsum_sb")
        nc.vector.tensor_add(outsum_sb[:], outk_sb[:, 0:V], outk_sb[:, V:2 * V])
        nc.sync.dma_start(out=outk_d.ap(), in_=outk_sb[:])
        nc.sync.dma_start(out=outsum_d.ap(), in_=outsum_sb[:])

    nc.compile()
    return nc


def prepare_inputs(audio_feats, image_feats, audio_cls, image_cls, audio_mask):
    """Host-side shard + layout prep. Returns (MT, in_maps)."""
    af = np.ascontiguousarray(audio_feats, dtype=np.float32).reshape(A, K, KC, KP, T)
    imf = np.ascontiguousarray(image_feats, dtype=np.float32).reshape(V, K, KC, KP, HW)
    acls = np.ascontiguousarray(audio_cls, dtype=np.float32).reshape(A, K, KC, KP)
    icls = np.ascontiguousarray(image_cls, dtype=np.float32).reshape(V, K, KC, KP)
    mask = np.asarray(audio_mask)
    maskf = mask.astype(np.float32)

    # image features, shared by all cores: (K, KC, KP, V*HW)
    imf_h = np.ascontiguousarray(imf.transpose(1, 2, 3, 0, 4).reshape(K, KC, KP, V * HW))
    icls_h = np.ascontiguousarray(icls.transpose(1, 2, 3, 0))

    per_core = []
    for ci in range(NCORES):
        asl = slice(ci * AL, (ci + 1) * AL)
        m_core = mask[asl]  # (AL, T)
        if GATHER:
            rows_j, rows_t = np.nonzero(m_core != 0)
            mvals = np.ones(len(rows_j), np.float32)
        else:
            rows_j, rows_t = np.indices((AL, T)).reshape(2, -1)
            mvals = maskf[asl][rows_j, rows_t]
        per_core.append((asl, rows_j, rows_t, mvals))

    L_max = max(len(pc[1]) for pc in per_core)
    MT = max(1, math.ceil(L_max / 128))
    LP = MT * 128

    in_maps = []
    for asl, rows_j, rows_t, mvals in per_core:
        L = len(rows_j)
        af_core = af[asl]  # (AL, K, KC, KP, T)
        af_rows = np.zeros((LP, K, KC, KP), np.float32)
        af_rows[:L] = af_core[rows_j, :, :, :, rows_t]
        afp = np.ascontiguousarray(
            af_rows.transpose(1, 2, 3, 0).reshape(K, KC, KP, MT * 128)
        )
        oh = np.zeros((LP, AL), np.float32)
        oh[np.arange(L), rows_j] = 1.0
        onehot = np.ascontiguousarray(oh.reshape(MT, 128, AL).transpose(0, 2, 1))
        mc = np.zeros((LP, AL), np.float32)
        mc[np.arange(L), rows_j] = mvals
        maskc = np.ascontiguousarray(mc.reshape(MT, 128, AL))
        in_maps.append({
            "afp": afp,
            "imf": imf_h,
            "acls": np.ascontiguousarray(acls[asl].transpose(1, 2, 3, 0)),
            "icls": icls_h,
            "onehot": onehot,
            "maskc": maskc,
            "maskf": np.ascontiguousarray(maskf[asl]),
        })
    return MT, in_maps


def get_program(MT: int):
    if MT not in _kernel_cache:
        _kernel_cache[MT] = _build(MT)
    return _kernel_cache[MT]


def kernel(audio_feats, image_feats, audio_cls, image_cls, audio_mask, agg_heads):
    global LAST_RESULTS
    MT, in_maps = prepare_inputs(
        audio_feats, image_feats, audio_cls, image_cls, audio_mask
    )
    nc = get_program(MT)
    res = run_bass_kernel_spmd(nc, in_maps, list(range(NCORES)), trace=TRACE)
    LAST_RESULTS = res
    agg = bool(np.asarray(agg_heads))
    outs = []
    for ci in range(NCORES):
        if agg:
            outs.append(res.results[ci]["outsum"])  # (AL, V)
        else:
            outk = res.results[ci]["outk"].reshape(AL, K, V)
            outs.append(outk.transpose(0, 2, 1))    # (AL, V, K)
    return np.concatenate(outs, axis=0).astype(np.float32)


# revision 5
# speedup vs baseline: 1.1518x; 1.1518x over previous
"""Trainium2 Bass kernel for nn_BaseAggregator_31439160607279.

Math (reference):
  af (a,c,f,t), imf (v,c,h,w), split c into k=2 heads of 256 ch.
  sims[a,v,k,hw,t] = sum_c af*imf ; + cls[a,v,k] ; relu ; max over hw ;
  masked mean over t (mask m[a,t] in {0,1}, den = f*sum_t m) ; sum over k.

Strategy:
  - Shard the audio dim a=32 across 8 cores (4 audios/core); images replicated.
  - Pack (a_local, t) pairs into the matmul M dim, keeping only mask-active t
    (m=0 columns contribute nothing to the masked sum).
  - Big matmuls in float32r (fp32 @ ~FP22, full PE rate at N>=256):
      lhsT = af[a,k] channel-chunk (K=128, M=128 packed rows)
      rhs  = [imf[v0,k] | imf[v1,k]] (K=128, N=392), accumulate 2 chunks.
  - relu(max_hw(x)+cls) == max_hw(relu(x+cls)): reduce_max on raw PSUM,
    then add per-(row,a)-broadcast cls (one-hot matmul) and relu.
  - Masked t-sum via matmul with block-diagonal mask columns (K=packed rows,
    M=4 audios), accumulated over M-tiles; divide by den, sum heads.
"""

import math
from contextlib import ExitStack

import numpy as np

import concourse.bacc as bacc
import concourse.mybir as mybir
import concourse.tile as tile
from concourse.bass_utils import run_bass_kernel_spmd

# Problem dims (hardcoded per spec)
A, V, C, F, T, H, W = 32, 32, 512, 1, 200, 14, 14
K = 2                    # heads
NCH = C // K             # 256 channels per head
KC = 2                   # channel chunks per head
KP = NCH // KC           # 128 = contraction per matmul
HW = H * W               # 196
NCORES = 8
AL = A // NCORES         # 4 local audios per core
NVP = V // 2             # 16 image pairs
NPAIR = 2 * HW           # 392 = matmul free dim

GATHER = True            # pack only mask-active (a, t) rows
DMA_G = 4                # image pairs per DMA chunk

TRACE = False            # set True (e.g. from test.py) to profile
LAST_RESULTS = None      # BassKernelResults of the last run

_kernel_cache = {}

f32 = mybir.dt.float32
f32r = mybir.dt.float32r
f16 = mybir.dt.float16
X = mybir.AxisListType.X


def _build(MT: int):
    """Build + compile the per-core Bass program for MT packed-row tiles."""
    nc = bacc.Bacc("TRN2", target_bir_lowering=False, debug=False)

    afp_d = nc.dram_tensor("afp", (K, KC, KP, MT * 128), f16, kind="ExternalInput")
    imf_d = nc.dram_tensor("imf", (K, KC, KP, V * HW), f16, kind="ExternalInput")
    acls_d = nc.dram_tensor("acls", (K, KC, KP, AL), f32, kind="ExternalInput")
    icls_d = nc.dram_tensor("icls", (K, KC, KP, V), f32, kind="ExternalInput")
    onehot_d = nc.dram_tensor("onehot", (MT, AL, 128), f32, kind="ExternalInput")
    maskc_d = nc.dram_tensor("maskc", (MT, 128, AL), f32, kind="ExternalInput")
    maskf_d = nc.dram_tensor("maskf", (AL, T), f32, kind="ExternalInput")
    outk_d = nc.dram_tensor("outk", (AL, K * V), f32, kind="ExternalOutput")
    outsum_d = nc.dram_tensor("outsum", (AL, V), f32, kind="ExternalOutput")

    with tile.TileContext(nc) as tc, ExitStack() as ctx:
        cst = ctx.enter_context(tc.tile_pool(name="cst", bufs=1))
        ps_big = ctx.enter_context(tc.tile_pool(name="ps_big", bufs=3, space="PSUM"))
        ps_sm = ctx.enter_context(tc.tile_pool(name="ps_sm", bufs=2, space="PSUM"))

        # --- persistent SBUF tiles; DMA order tuned so compute starts early ---
        afp_sb, imf_sb, acls_sb, icls_sb = {}, {}, {}, {}
        for k in range(K):
            for kc in range(KC):
                afp_sb[k, kc] = cst.tile([KP, MT * 128], f16, tag=f"afp{k}{kc}", name=f"afp{k}{kc}")
                acls_sb[k, kc] = cst.tile([KP, AL], f32, tag=f"acls{k}{kc}", name=f"acls{k}{kc}")
                icls_sb[k, kc] = cst.tile([KP, V], f32, tag=f"icls{k}{kc}", name=f"icls{k}{kc}")
                imf_sb[k, kc] = cst.tile([KP, V * HW], f16, tag=f"imf{k}{kc}", name=f"imf{k}{kc}")

        # 1) tiny cls inputs (gate the first PE instructions)
        for k in range(K):
            for kc in range(KC):
                nc.sync.dma_start(out=acls_sb[k, kc][:], in_=acls_d.ap()[k, kc])
                nc.sync.dma_start(out=icls_sb[k, kc][:], in_=icls_d.ap()[k, kc])
        # 2) audio rows + first image chunk for k=0
        for kc in range(KC):
            nc.sync.dma_start(out=afp_sb[0, kc][:], in_=afp_d.ap()[0, kc])
        g0 = slice(0, DMA_G * NPAIR)
        for kc in range(KC):
            nc.sync.dma_start(out=imf_sb[0, kc][:, g0], in_=imf_d.ap()[0, kc][:, g0])
        for kc in range(KC):
            nc.sync.dma_start(out=afp_sb[1, kc][:], in_=afp_d.ap()[1, kc])
        # 3) small aux tensors
        onehot_sb, maskc_sb = [], []
        for mt in range(MT):
            toh = cst.tile([AL, 128], f32, tag=f"onehot{mt}", name=f"onehot{mt}")
            nc.sync.dma_start(out=toh[:], in_=onehot_d.ap()[mt])
            onehot_sb.append(toh)
            tmc = cst.tile([128, AL], f32, tag=f"maskc{mt}", name=f"maskc{mt}")
            nc.sync.dma_start(out=tmc[:], in_=maskc_d.ap()[mt])
            maskc_sb.append(tmc)
        maskf_sb = cst.tile([AL, T], f32, tag="maskf", name="maskf_sb")
        nc.sync.dma_start(out=maskf_sb[:], in_=maskf_d.ap())
        # 4) remaining image chunks in compute order
        for k in range(K):
            for g in range(NVP // DMA_G):
                if k == 0 and g == 0:
                    continue
                sl = slice(g * DMA_G * NPAIR, (g + 1) * DMA_G * NPAIR)
                for kc in range(KC):
                    nc.sync.dma_start(
                        out=imf_sb[k, kc][:, sl], in_=imf_d.ap()[k, kc][:, sl]
                    )

        # --- cls_sims: cls[a, k*V+v] = sum_c acls*icls ---
        cls_sb = cst.tile([AL, K * V], f32, tag="cls", name="cls_sb")
        for k in range(K):
            pc = ps_sm.tile([AL, V], f32, tag="ps_sm", name="ps_cls")
            for kc in range(KC):
                nc.tensor.matmul(
                    pc[:], lhsT=acls_sb[k, kc][:], rhs=icls_sb[k, kc][:],
                    start=(kc == 0), stop=(kc == 1),
                )
            nc.vector.tensor_copy(cls_sb[:, k * V:(k + 1) * V], pc[:])

        den = cst.tile([AL, 1], f32, tag="den", name="den")
        nc.vector.reduce_sum(den[:], maskf_sb[:], axis=X)
        rden = cst.tile([AL, 1], f32, tag="rden", name="rden")
        nc.vector.reciprocal(rden[:], den[:])

        # --- main loop: sims matmuls + max over hw (2 image pairs per PSUM) ---
        smraw = [cst.tile([128, K * V], f32, tag=f"smraw{mt}", name=f"smraw{mt}") for mt in range(MT)]
        for k in range(K):
            for vg in range(NVP // 2):
                col = k * V + vg * 4
                for mt in range(MT):
                    ps = ps_big.tile([128, 1024], f32, tag="ps_big", name="ps_sims")
                    for sub in range(2):
                        vp = vg * 2 + sub
                        rsl = slice(vp * NPAIR, (vp + 1) * NPAIR)
                        for kc in range(KC):
                            nc.tensor.matmul(
                                ps[:, sub * 512:sub * 512 + NPAIR],
                                lhsT=afp_sb[k, kc][:, mt * 128:(mt + 1) * 128],
                                rhs=imf_sb[k, kc][:, rsl],
                                start=(kc == 0), stop=(kc == 1),
                            )
                    rview = ps[:].rearrange("p (b q) -> p b q", b=2)[:, :, 0:NPAIR]
                    rview = rview.rearrange("p b (i x) -> p b i x", i=2)
                    nc.vector.reduce_max(smraw[mt][:, col:col + 4], rview, axis=X)

        # --- per M-tile: +cls, relu, masked row-sum ---
        num_sb = cst.tile([AL, K * V], f32, tag="num", name="num_sb")
        nc.vector.memset(num_sb[:], 0.0)
        sm_pool = ctx.enter_context(tc.tile_pool(name="sm", bufs=2))
        for mt in range(MT):
            pb = ps_sm.tile([128, K * V], f32, tag="ps_sm", name="ps_bcast")
            nc.tensor.matmul(pb[:], lhsT=onehot_sb[mt][:], rhs=cls_sb[:],
                             start=True, stop=True)
            sm2 = sm_pool.tile([128, K * V], f32, tag="sm2", name="sm2")
            nc.vector.tensor_add(sm2[:], smraw[mt][:], pb[:])
            nc.vector.tensor_scalar_max(sm2[:], sm2[:], 0.0)
            pn = ps_sm.tile([AL, K * V], f32, tag="ps_sm", name="ps_num")
            nc.tensor.matmul(pn[:], lhsT=maskc_sb[mt][:], rhs=sm2[:],
                             start=True, stop=True)
            nc.vector.tensor_add(num_sb[:], num_sb[:], pn[:])

        # --- divide, head-sum, out ---
        outk_sb = cst.tile([AL, K * V], f32, tag="outk", name="outk_sb")
        nc.vector.tensor_scalar_mul(outk_sb[:], num_sb[:], rden[:])
        outsum_sb = cst.tile([AL, V], f32, tag="outsum", name="outsum_sb")
        nc.vector.tensor_add(outsum_sb[:], outk_sb[:, 0:V], outk_sb[:, V:2 * V])
        nc.sync.dma_start(out=outk_d.ap(), in_=outk_sb[:])
        nc.sync.dma_start(out=outsum_d.ap(), in_=outsum_sb[:])

    nc.compile()
    return nc


def prepare_inputs(audio_feats, image_feats, audio_cls, image_cls, audio_mask):
    """Host-side shard + layout prep. Returns (MT, in_maps)."""
    af = np.ascontiguousarray(audio_feats, dtype=np.float32).reshape(A, K, KC, KP, T)
    imf = np.ascontiguousarray(image_feats, dtype=np.float32).reshape(V, K, KC, KP, HW)
    acls = np.ascontiguousarray(audio_cls, dtype=np.float32).reshape(A, K, KC, KP)
    icls = np.ascontiguousarray(image_cls, dtype=np.float32).reshape(V, K, KC, KP)
    mask = np.asarray(audio_mask)
    maskf = mask.astype(np.float32)

    # image features, shared by all cores: (K, KC, KP, V*HW)
    imf_h = np.ascontiguousarray(imf.transpose(1, 2, 3, 0, 4).reshape(K, KC, KP, V * HW)).astype(np.float16)
    icls_h = np.ascontiguousarray(icls.transpose(1, 2, 3, 0))

    per_core = []
    for ci in range(NCORES):
        asl = slice(ci * AL, (ci + 1) * AL)
        m_core = mask[asl]  # (AL, T)
        if GATHER:
            rows_j, rows_t = np.nonzero(m_core != 0)
            mvals = np.ones(len(rows_j), np.float32)
        else:
            rows_j, rows_t = np.indices((AL, T)).reshape(2, -1)
            mvals = maskf[asl][rows_j, rows_t]
        per_core.append((asl, rows_j, rows_t, mvals))

    L_max = max(len(pc[1]) for pc in per_core)
    MT = max(1, math.ceil(L_max / 128))
    LP = MT * 128

    in_maps = []
    for asl, rows_j, rows_t, mvals in per_core:
        L = len(rows_j)
        af_core = af[asl]  # (AL, K, KC, KP, T)
        af_rows = np.zeros((LP, K, KC, KP), np.float32)
        af_rows[:L] = af_core[rows_j, :, :, :, rows_t]
        afp = np.ascontiguousarray(
            af_rows.transpose(1, 2, 3, 0).reshape(K, KC, KP, MT * 128)
        ).astype(np.float16)
        oh = np.zeros((LP, AL), np.float32)
        oh[np.arange(L), rows_j] = 1.0
        onehot = np.ascontiguousarray(oh.reshape(MT, 128, AL).transpose(0, 2, 1))
        mc = np.zeros((LP, AL), np.float32)
        mc[np.arange(L), rows_j] = mvals
        maskc = np.ascontiguousarray(mc.reshape(MT, 128, AL))
        in_maps.append({
            "afp": afp,
            "imf": imf_h,
            "acls": np.ascontiguousarray(acls[asl].transpose(1, 2, 3, 0)),
            "icls": icls_h,
            "onehot": onehot,
            "maskc": maskc,
            "maskf": np.ascontiguousarray(maskf[asl]),
        })
    return MT, in_maps


def get_program(MT: int):
    if MT not in _kernel_cache:
        _kernel_cache[MT] = _build(MT)
    return _kernel_cache[MT]


def kernel(audio_feats, image_feats, audio_cls, image_cls, audio_mask, agg_heads):
    global LAST_RESULTS
    MT, in_maps = prepare_inputs(
        audio_feats, image_feats, audio_cls, image_cls, audio_mask
    )
    nc = get_program(MT)
    res = run_bass_kernel_spmd(nc, in_maps, list(range(NCORES)), trace=TRACE)
    LAST_RESULTS = res
    agg = bool(np.asarray(agg_heads))
    outs = []
    for ci in range(NCORES):
        if agg:
            outs.append(res.results[ci]["outsum"])  # (AL, V)
        else:
            outk = res.results[ci]["outk"].reshape(AL, K, V)
            outs.append(outk.transpose(0, 2, 1))    # (AL, V, K)
    return np.concatenate(outs, axis=0).astype(np.float32)
